# revision 2
# baseline (speedup 1.0000x reference)
"""Multi-head attention (B=4, T=1024, D=1024, H=16) on 8 TRN2 NeuronCores.

Sharding: core c = (batch b=c//2, head-group g=c%2); each core computes 8 heads
of one batch (column-parallel qkv, row-parallel out_proj). Host sums the two
partial out-projections per batch (the row-parallel reduce).

Per-core dataflow (all matmuls bf16 inputs, fp32 PSUM accumulate):
  qT,kT = (Wq|Wk).T @ x        -> [128(2 heads*64), T] transposed layout
  v     = x @ Wv               -> [T, 512] natural layout (+ ones col per head)
  S^T   = kT.T @ qT tiles      -> [128 j, 512 i] per (head, j-tile, q-group)
  P^T   = exp(S^T + maskT)     -> bf16 (causal upper tiles skipped, -1e30 mask)
  O^T   = V'.T @ P^T           -> [65, 512]: rows 0-63 = out, row 64 = rowsum
  outT  = O^T[0:64] * (1/rowsum broadcast)
  final = outT.T @ Wout rows   -> partial [T, D], summed across head-groups on host
"""

import numpy as np
import ml_dtypes

import concourse.bass as bass
import concourse.tile as tile
from concourse import bacc, mybir
from concourse.bass_utils import run_bass_kernel_spmd

B, T, D, H = 4, 1024, 1024, 16
HD = D // H            # 64
HPC = 8                # heads per core
N_CORES = 8
BF16 = mybir.dt.bfloat16
F32 = mybir.dt.float32
NEG = -1.0e30

_cached_nc = None


def build_nc():
    nc = bacc.Bacc("TRN2", target_bir_lowering=False, debug=False)

    xT = nc.dram_tensor("xT", [D, T], BF16, kind="ExternalInput").ap()
    wqk = nc.dram_tensor("wqk", [D, 1024], BF16, kind="ExternalInput").ap()
    wv = nc.dram_tensor("wv", [D, 512], BF16, kind="ExternalInput").ap()
    wout = nc.dram_tensor("wout", [512, D], BF16, kind="ExternalInput").ap()
    maskt = nc.dram_tensor("maskt", [8, 128, 512], BF16, kind="ExternalInput").ap()
    out = nc.dram_tensor("out", [T, D], F32, kind="ExternalOutput").ap()

    with tile.TileContext(nc) as tc:
        with (
            tc.tile_pool(name="persist", bufs=1) as pp,
            tc.tile_pool(name="work", bufs=4) as wp,
            tc.tile_pool(name="ps_a", bufs=2, space="PSUM") as ps_a,
            tc.tile_pool(name="ps_s", bufs=2, space="PSUM") as ps_s,
            tc.tile_pool(name="ps_o", bufs=2, space="PSUM") as ps_o,
            tc.tile_pool(name="ps_f", bufs=2, space="PSUM") as ps_f,
        ):
            # ---- load everything resident to SBUF ----
            xT_sb = []
            wqk_sb = []
            wv_sb = []
            for kc in range(8):
                t = pp.tile([128, T], BF16, name=f"xt{kc}")
                nc.sync.dma_start(out=t, in_=xT[kc * 128:(kc + 1) * 128, :])
                xT_sb.append(t)
            for kc in range(8):
                t = pp.tile([128, 1024], BF16, name=f"wqk{kc}")
                nc.sync.dma_start(out=t, in_=wqk[kc * 128:(kc + 1) * 128, :])
                wqk_sb.append(t)
            for kc in range(8):
                t = pp.tile([128, 512], BF16, name=f"wv{kc}")
                nc.sync.dma_start(out=t, in_=wv[kc * 128:(kc + 1) * 128, :])
                wv_sb.append(t)
            wout_sb = []
            for kc in range(4):
                t = pp.tile([128, D], BF16, name=f"wout{kc}")
                nc.sync.dma_start(out=t, in_=wout[kc * 128:(kc + 1) * 128, :])
                wout_sb.append(t)
            maskt_sb = pp.tile([128, 8, 512], BF16, name="maskt")
            nc.sync.dma_start(
                out=maskt_sb, in_=maskt.rearrange("j p f -> p j f")
            )

            # ---- stage A: v = x @ Wv (natural), augmented with ones column ----
            v_aug = []
            for tt in range(8):
                t = pp.tile([128, HPC, 65], BF16, name=f"vaug{tt}")
                nc.vector.memset(t[:, :, 64:65], 1.0)
                v_aug.append(t)
            for tt in range(8):
                ps = ps_a.tile([128, 512], F32, name="ps_a")
                for kc in range(8):
                    nc.tensor.matmul(
                        ps,
                        lhsT=xT_sb[kc][:, tt * 128:(tt + 1) * 128],
                        rhs=wv_sb[kc],
                        start=(kc == 0),
                        stop=(kc == 7),
                    )
                nc.vector.tensor_copy(
                    out=v_aug[tt][:, :, 0:64],
                    in_=ps.rearrange("p (h e) -> p h e", e=64),
                )

            # ---- stage A: qT / kT = (Wq|Wk).T @ x (transposed layout) ----
            # chunk 0..3 = q (head pair 2c,2c+1), chunk 4..7 = k
            qkT_sb = []
            for ch in range(8):
                qkT_sb.append(pp.tile([128, T], BF16, name=f"qkt{ch}"))
            for ch in range(8):
                for tg in range(2):
                    ps = ps_a.tile([128, 512], F32, name="ps_a")
                    for kc in range(8):
                        nc.tensor.matmul(
                            ps,
                            lhsT=wqk_sb[kc][:, ch * 128:(ch + 1) * 128],
                            rhs=xT_sb[kc][:, tg * 512:(tg + 1) * 512],
                            start=(kc == 0),
                            stop=(kc == 7),
                        )
                    nc.vector.tensor_copy(
                        out=qkT_sb[ch][:, tg * 512:(tg + 1) * 512], in_=ps
                    )

            # ---- attention, head pair c = local heads (2c, 2c+1) ----
            outT_sb = []
            for ch in range(4):
                outT_sb.append(pp.tile([128, T], BF16, name=f"outt{ch}"))
            for c in range(4):
                qt = qkT_sb[c]
                kt = qkT_sb[4 + c]
                for grp in range(2):
                    jmax = 4 * grp + 3
                    po = [
                        ps_o.tile([65, 512], F32, name="ps_o"),
                        ps_o.tile([65, 512], F32, name="ps_o"),
                    ]
                    for jt in range(jmax + 1):
                        for h01 in range(2):
                            r0 = h01 * 64
                            ps = ps_s.tile([128, 512], F32, name="ps_s")
                            nc.tensor.matmul(
                                ps,
                                lhsT=kt[r0:r0 + 64, jt * 128:(jt + 1) * 128],
                                rhs=qt[r0:r0 + 64, grp * 512:(grp + 1) * 512],
                                start=True,
                                stop=True,
                            )
                            p_sb = wp.tile([128, 512], BF16, name="p_sb")
                            if jt // 4 == grp:
                                m_sb = wp.tile([128, 512], F32, name="m_sb")
                                nc.vector.tensor_add(
                                    m_sb, ps, maskt_sb[:, jt, :]
                                )
                                nc.scalar.activation(
                                    p_sb, m_sb, mybir.ActivationFunctionType.Exp
                                )
                            else:
                                nc.scalar.activation(
                                    p_sb, ps, mybir.ActivationFunctionType.Exp
                                )
                            h = 2 * c + h01
                            nc.tensor.matmul(
                                po[h01],
                                lhsT=v_aug[jt][:, h, :],
                                rhs=p_sb,
                                start=(jt == 0),
                                stop=(jt == jmax),
                            )
                    for h01 in range(2):
                        rec = wp.tile([1, 512], F32, name="rec")
                        nc.vector.reciprocal(rec, po[h01][64:65, :])
                        rec_b = wp.tile([64, 512], F32, name="rec_b")
                        nc.gpsimd.partition_broadcast(rec_b, rec)
                        nc.vector.tensor_mul(
                            outT_sb[c][h01 * 64:(h01 + 1) * 64,
                                       grp * 512:(grp + 1) * 512],
                            po[h01][0:64, :],
                            rec_b,
                        )

            # ---- out_proj: final[t, n] = sum_ch outT[ch].T @ wout[ch] ----
            for it in range(8):
                for ng in range(2):
                    ps = ps_f.tile([128, 512], F32, name="ps_f")
                    for kc in range(4):
                        nc.tensor.matmul(
                            ps,
                            lhsT=outT_sb[kc][:, it * 128:(it + 1) * 128],
                            rhs=wout_sb[kc][:, ng * 512:(ng + 1) * 512],
                            start=(kc == 0),
                            stop=(kc == 3),
                        )
                    o_sb = wp.tile([128, 512], F32, name="o_sb")
                    nc.vector.tensor_copy(out=o_sb, in_=ps)
                    nc.sync.dma_start(
                        out=out[it * 128:(it + 1) * 128,
                                ng * 512:(ng + 1) * 512],
                        in_=o_sb,
                    )

    nc.compile()
    return nc


def _prep_inputs(x, mask, w_qkv, w_out):
    bf = ml_dtypes.bfloat16
    x = np.asarray(x, dtype=np.float32)
    mask = np.asarray(mask)
    w_qkv = np.asarray(w_qkv, dtype=np.float32)
    w_out = np.asarray(w_out, dtype=np.float32)

    wq = w_qkv[:, 0:D]
    wk = w_qkv[:, D:2 * D]
    wv = w_qkv[:, 2 * D:3 * D]

    madd = np.where(mask[0, 0] == 1, np.float32(NEG), np.float32(0.0))
    maskT = madd.T  # [k, q]
    mtiles = np.stack(
        [maskT[jt * 128:(jt + 1) * 128,
               (jt // 4) * 512:(jt // 4 + 1) * 512] for jt in range(8)]
    ).astype(bf)

    in_maps = []
    for c in range(N_CORES):
        b, g = divmod(c, 2)
        cols = slice(g * 512, (g + 1) * 512)
        wqk_c = np.concatenate(
            [wq[:, cols] * np.float32(1.0 / np.sqrt(HD)), wk[:, cols]], axis=1
        ).astype(bf)
        in_maps.append({
            "xT": np.ascontiguousarray(x[b].T).astype(bf),
            "wqk": wqk_c,
            "wv": wv[:, cols].astype(bf),
            "wout": w_out[g * 512:(g + 1) * 512, :].astype(bf),
            "maskt": mtiles,
        })
    return in_maps


def kernel(x, mask, w_qkv, w_out):
    global _cached_nc
    if _cached_nc is None:
        _cached_nc = build_nc()
    in_maps = _prep_inputs(x, mask, w_qkv, w_out)
    res = run_bass_kernel_spmd(_cached_nc, in_maps, core_ids=list(range(N_CORES)))
    outs = [res.results[c]["out"] for c in range(N_CORES)]
    full = np.empty((B, T, D), dtype=np.float32)
    for b in range(B):
        full[b] = outs[2 * b] + outs[2 * b + 1]
    return full


# revision 4
# speedup vs baseline: 1.5122x; 1.5122x over previous
"""Multi-head attention (B=4, T=1024, D=1024, H=16) on 8 TRN2 NeuronCores.

Sharding: core c = (batch b=c//2, head-group g=c%2); each core computes 8 heads
of one batch (column-parallel qkv, row-parallel out_proj). Host sums the two
partial out-projections per batch (the row-parallel reduce).

Per-core dataflow (bf16 matmul inputs, fp32 PSUM accumulate):
  qT,kT = (Wq|Wk).T @ x        -> [128(2 heads*64), T] transposed layout
  v     = x @ Wv               -> [T, 512] natural layout (+ ones col per head)
  S^T   = kT.T @ qT tiles      -> [128 j, 512 i] per (head, j-tile, q-group)
          + I @ maskd accumulated into the diagonal 128-window by the PE
  P^T   = exp(S^T) on the causal window, left region memset to 0
  O^T   = V'.T @ P^T           -> [65, 512]: rows 0-63 = out, row 64 = rowsum
  outT  = O^T[0:64] * (1/rowsum broadcast)
  final = outT.T @ Wout rows   -> partial [T, D], summed across head-groups on host
"""

import numpy as np
import ml_dtypes

import concourse.bass as bass
import concourse.tile as tile
from concourse import bacc, mybir
from concourse.bass_utils import run_bass_kernel_spmd
from concourse.masks import make_identity

B, T, D, H = 4, 1024, 1024, 16
HD = D // H            # 64
HPC = 8                # heads per core
N_CORES = 8
BF16 = mybir.dt.bfloat16
F32 = mybir.dt.float32
NEG = -1.0e30

_cached_nc = None


def build_nc():
    nc = bacc.Bacc("TRN2", target_bir_lowering=False, debug=False)

    xT = nc.dram_tensor("xT", [D, T], BF16, kind="ExternalInput").ap()
    wqk = nc.dram_tensor("wqk", [D, 1024], BF16, kind="ExternalInput").ap()
    wv = nc.dram_tensor("wv", [D, 512], BF16, kind="ExternalInput").ap()
    wout = nc.dram_tensor("wout", [512, D], BF16, kind="ExternalInput").ap()
    maskd = nc.dram_tensor("maskd", [128, 128], BF16, kind="ExternalInput").ap()
    out = nc.dram_tensor("out", [T, D], F32, kind="ExternalOutput").ap()

    EXP = mybir.ActivationFunctionType.Exp

    with tile.TileContext(nc) as tc:
        with (
            tc.tile_pool(name="persist", bufs=1) as pp,
            tc.tile_pool(name="work", bufs=4) as wp,
            tc.tile_pool(name="ps_af", bufs=2, space="PSUM") as ps_af,
            tc.tile_pool(name="ps_s", bufs=3, space="PSUM") as ps_s,
            tc.tile_pool(name="ps_o", bufs=3, space="PSUM") as ps_o,
        ):
            # ---- resident loads ----
            xT_sb, wqk_sb, wv_sb, wout_sb = [], [], [], []
            for kc in range(8):
                t = pp.tile([128, T], BF16, name=f"xt{kc}")
                nc.sync.dma_start(out=t, in_=xT[kc * 128:(kc + 1) * 128, :])
                xT_sb.append(t)
            for kc in range(8):
                t = pp.tile([128, 1024], BF16, name=f"wqk{kc}")
                nc.sync.dma_start(out=t, in_=wqk[kc * 128:(kc + 1) * 128, :])
                wqk_sb.append(t)
            for kc in range(8):
                t = pp.tile([128, 512], BF16, name=f"wv{kc}")
                nc.sync.dma_start(out=t, in_=wv[kc * 128:(kc + 1) * 128, :])
                wv_sb.append(t)
            for kc in range(4):
                t = pp.tile([128, D], BF16, name=f"wout{kc}")
                nc.sync.dma_start(out=t, in_=wout[kc * 128:(kc + 1) * 128, :])
                wout_sb.append(t)
            maskd_sb = pp.tile([128, 128], BF16, name="maskd")
            nc.sync.dma_start(out=maskd_sb, in_=maskd)
            ident = pp.tile([128, 128], BF16, name="ident")
            make_identity(nc, ident)

            # ---- v = x @ Wv (natural), ones column per head ----
            v_aug = []
            for tt in range(8):
                t = pp.tile([128, HPC, 65], BF16, name=f"vaug{tt}")
                nc.vector.memset(t[:, :, 64:65], 1.0)
                v_aug.append(t)
            for tt in range(8):
                ps = ps_af.tile([128, 512], F32, name="ps_af")
                for kc in range(8):
                    nc.tensor.matmul(
                        ps,
                        lhsT=xT_sb[kc][:, tt * 128:(tt + 1) * 128],
                        rhs=wv_sb[kc],
                        start=(kc == 0),
                        stop=(kc == 7),
                    )
                nc.vector.tensor_copy(
                    out=v_aug[tt][:, :, 0:64],
                    in_=ps.rearrange("p (h e) -> p h e", e=64),
                )

            # ---- per head-pair: qT/kT chunks then attention ----
            qkT_sb = [pp.tile([128, T], BF16, name=f"qkt{ch}") for ch in range(8)]
            outT_sb = [pp.tile([128, T], BF16, name=f"outt{ch}") for ch in range(4)]

            for c in range(4):
                for ch in (c, 4 + c):  # q chunk, k chunk
                    for tg in range(2):
                        ps = ps_af.tile([128, 512], F32, name="ps_af")
                        for kc in range(8):
                            nc.tensor.matmul(
                                ps,
                                lhsT=wqk_sb[kc][:, ch * 128:(ch + 1) * 128],
                                rhs=xT_sb[kc][:, tg * 512:(tg + 1) * 512],
                                start=(kc == 0),
                                stop=(kc == 7),
                            )
                        nc.vector.tensor_copy(
                            out=qkT_sb[ch][:, tg * 512:(tg + 1) * 512], in_=ps
                        )

                qt = qkT_sb[c]
                kt = qkT_sb[4 + c]
                for grp in range(2):
                    jmax = 4 * grp + 3
                    po = [
                        ps_o.tile([65, 512], F32, name="ps_o"),
                        ps_o.tile([65, 512], F32, name="ps_o"),
                    ]
                    for jt in range(jmax + 1):
                        diag = (jt // 4 == grp)
                        L = jt * 128 - grp * 512 if diag else 0
                        for h01 in range(2):
                            r0 = h01 * 64
                            ps = ps_s.tile([128, 512], F32, name="ps_s")
                            nc.tensor.matmul(
                                ps,
                                lhsT=kt[r0:r0 + 64, jt * 128:(jt + 1) * 128],
                                rhs=qt[r0:r0 + 64, grp * 512:(grp + 1) * 512],
                                start=True,
                                stop=not diag,
                            )
                            p_sb = wp.tile([128, 512], BF16, name="p_sb")
                            if diag:
                                nc.tensor.matmul(
                                    ps[:, L:L + 128],
                                    lhsT=ident,
                                    rhs=maskd_sb,
                                    start=False,
                                    stop=True,
                                )
                                if L > 0:
                                    nc.gpsimd.memset(p_sb[:, 0:L], 0.0)
                                nc.scalar.activation(
                                    p_sb[:, L:512], ps[:, L:512], EXP)
                            else:
                                nc.scalar.activation(p_sb, ps, EXP)
                            h = 2 * c + h01
                            nc.tensor.matmul(
                                po[h01],
                                lhsT=v_aug[jt][:, h, :],
                                rhs=p_sb,
                                start=(jt == 0),
                                stop=(jt == jmax),
                            )
                    for h01 in range(2):
                        rs_sb = wp.tile([1, 512], F32, name="rs_sb")
                        nc.vector.tensor_copy(out=rs_sb, in_=po[h01][64:65, :])
                        rec = wp.tile([1, 512], F32, name="rec")
                        nc.vector.reciprocal_approx_fast(out=rec, in_=rs_sb)
                        rec_b = wp.tile([64, 512], F32, name="rec_b")
                        nc.gpsimd.partition_broadcast(rec_b, rec)
                        nc.vector.tensor_mul(
                            outT_sb[c][h01 * 64:(h01 + 1) * 64,
                                       grp * 512:(grp + 1) * 512],
                            po[h01][0:64, :],
                            rec_b,
                        )

            # ---- out_proj ----
            for it in range(8):
                for ng in range(2):
                    ps = ps_af.tile([128, 512], F32, name="ps_af")
                    for kc in range(4):
                        nc.tensor.matmul(
                            ps,
                            lhsT=outT_sb[kc][:, it * 128:(it + 1) * 128],
                            rhs=wout_sb[kc][:, ng * 512:(ng + 1) * 512],
                            start=(kc == 0),
                            stop=(kc == 3),
                        )
                    o_sb = wp.tile([128, 512], F32, name="o_sb")
                    nc.vector.tensor_copy(out=o_sb, in_=ps)
                    nc.sync.dma_start(
                        out=out[it * 128:(it + 1) * 128,
                                ng * 512:(ng + 1) * 512],
                        in_=o_sb,
                    )

    nc.compile()
    return nc


def _prep_inputs(x, mask, w_qkv, w_out):
    bf = ml_dtypes.bfloat16
    x = np.asarray(x, dtype=np.float32)
    mask = np.asarray(mask)
    w_qkv = np.asarray(w_qkv, dtype=np.float32)
    w_out = np.asarray(w_out, dtype=np.float32)

    wq = w_qkv[:, 0:D]
    wk = w_qkv[:, D:2 * D]
    wv = w_qkv[:, 2 * D:3 * D]

    madd = np.where(mask[0, 0] == 1, np.float32(NEG), np.float32(0.0))
    maskd = np.ascontiguousarray(madd.T[0:128, 0:128]).astype(bf)

    in_maps = []
    for c in range(N_CORES):
        b, g = divmod(c, 2)
        cols = slice(g * 512, (g + 1) * 512)
        wqk_c = np.concatenate(
            [wq[:, cols] * np.float32(1.0 / np.sqrt(HD)), wk[:, cols]], axis=1
        ).astype(bf)
        in_maps.append({
            "xT": np.ascontiguousarray(x[b].T).astype(bf),
            "wqk": wqk_c,
            "wv": wv[:, cols].astype(bf),
            "wout": w_out[g * 512:(g + 1) * 512, :].astype(bf),
            "maskd": maskd,
        })
    return in_maps


def kernel(x, mask, w_qkv, w_out):
    global _cached_nc
    if _cached_nc is None:
        _cached_nc = build_nc()
    in_maps = _prep_inputs(x, mask, w_qkv, w_out)
    res = run_bass_kernel_spmd(_cached_nc, in_maps, core_ids=list(range(N_CORES)))
    outs = [res.results[c]["out"] for c in range(N_CORES)]
    full = np.empty((B, T, D), dtype=np.float32)
    for b in range(B):
        full[b] = outs[2 * b] + outs[2 * b + 1]
    return full
